# revision 11
# baseline (speedup 1.0000x reference)
"""GNN message-passing (4 edge-convs + heads) on 8 Trainium2 NeuronCores.

See DESIGN.md. Highlights:
- Pair-sharding (core c owns edge pairs (e, e+E/2)) -> edge head fully local.
- Per-core edges sorted by source row, packed into 256-edge-slot windows of
  whole per-node runs (<=192 distinct nodes per window, host-verified).
- Feature-major per-edge compute; node-product gathers via GPSIMD ap_gather;
- scatter-mean: one-hot (x recip, folded) matmuls accumulate window sums into
  PSUM; copied to dense label table XD at static offsets; un-permuted to
  global node order by a per-chunk ap_gather; AllReduce across cores.
- fp16 matmul operands (fp32r-class accuracy), fp32 PSUM + gather tables.
"""
import numpy as np

import concourse.bass as bass
import concourse.mybir as mybir
import concourse.tile as tile
from concourse import bacc, library_config
from concourse.bass_utils import run_bass_kernel_spmd

LAST_RESULT = None

F32 = mybir.dt.float32
F16 = mybir.dt.float16
I16 = mybir.dt.int16
AF = mybir.ActivationFunctionType
ALU = mybir.AluOpType

NCORES = 8
T = 512          # edge tile (moving dim)
CH = 128         # scatter chunk (edges on partitions)
WE = 256         # edge slots per window (2 chunks)
WN = 192         # labels per window


def _wrap16(vals, ngroups, nslots):
    """ap_gather wrapped int16 layout: element j at [j%16, j//16], replicated
    across `ngroups` 16-partition groups."""
    v = np.asarray(vals, np.int64)
    n = v.shape[0]
    assert n <= nslots * 16
    out = np.zeros((ngroups * 16, nslots), np.int16)
    j = np.arange(n)
    for g in range(ngroups):
        out[g * 16 + (j % 16), j // 16] = v
    return out


def _host_prep(x_org, x_rot, edge_index, edge_attr, edge_rot, weights):
    N = x_org.shape[0]
    E = edge_index.shape[1]
    Eh = E // 2
    S = Eh // NCORES                     # pairs per core
    EC = 2 * S                           # edges per core
    NP = ((N + T - 1) // T) * T
    NPCH = NP // T

    row = np.asarray(edge_index[0], np.int64)
    col = np.asarray(edge_index[1], np.int64)
    cnt = np.bincount(row, minlength=N).astype(np.float64)
    assert (np.bincount(col, minlength=N) >= 0).all()
    recip = (1.0 / np.maximum(cnt, 1.0)).astype(np.float32)

    # ---- per-core: sort by row, pack node-runs into windows ----
    cores = []
    max_slots = 0
    for c in range(NCORES):
        ids = np.concatenate([np.arange(S * c, S * (c + 1)),
                              Eh + np.arange(S * c, S * (c + 1))])
        r = row[ids]
        order = np.argsort(r, kind="stable")
        ids_s = ids[order]
        r_s = r[order]
        # node runs (start, len)
        newn = np.empty(EC, np.bool_)
        newn[0] = True
        newn[1:] = r_s[1:] != r_s[:-1]
        starts = np.nonzero(newn)[0]
        lens = np.diff(np.append(starts, EC))
        assert lens.max() <= WE, "node degree exceeds window capacity"
        # greedy pack runs -> windows (capacity WE slots, WN labels)
        slot_of = np.empty(EC, np.int64)      # sorted-edge -> slot
        lab_of_run = np.empty(len(starts), np.int64)
        slot = 0
        wnd = 0
        used_slots = 0
        used_labs = 0
        for ri in range(len(starts)):
            L = lens[ri]
            if used_slots + L > WE or used_labs + 1 > WN:
                slot += WE - used_slots           # pad window tail
                wnd += 1
                used_slots = 0
                used_labs = 0
            lab_of_run[ri] = WN * wnd + used_labs
            slot_of[starts[ri]: starts[ri] + L] = slot + np.arange(L)
            slot += L
            used_slots += L
            used_labs += 1
        total_slots = (wnd + 1) * WE
        cores.append(dict(ids_s=ids_s, r_s=r_s, order=order, starts=starts,
                          lens=lens, slot_of=slot_of, lab_of_run=lab_of_run,
                          total_slots=total_slots))
        max_slots = max(max_slots, total_slots)

    EPAD = ((max_slots + T - 1) // T) * T
    NT = EPAD // T
    NCH = EPAD // CH
    NWIN = NCH // 2
    NLAB = NWIN * WN
    NLABP = NLAB + 64                    # dummy zero cols at NLAB..
    DUMMY = NLAB
    SP = ((S + T - 1) // T) * T

    iota = np.broadcast_to(np.arange(WN, dtype=np.float16), (128, WN)).copy()

    per_core = []
    for c in range(NCORES):
        cc = cores[c]
        ids_s, r_s, slot_of = cc["ids_s"], cc["r_s"], cc["slot_of"]
        c_s = col[ids_s]
        # slot-order arrays
        r_slot = np.zeros(EPAD, np.int64)
        c_slot = np.zeros(EPAD, np.int64)
        lab_slot = np.full(EPAD, -1, np.int64)
        occ = np.zeros(EPAD, np.bool_)
        r_slot[slot_of] = r_s
        c_slot[slot_of] = c_s
        occ[slot_of] = True
        lab_e = np.repeat(cc["lab_of_run"], cc["lens"])     # per sorted edge
        lab_slot[slot_of] = lab_e

        # one-hot inputs
        rr = np.full(EPAD, -1.0, np.float32)
        rr[occ] = lab_slot[occ] % WN
        rrec = np.zeros(EPAD, np.float32)
        rrec[slot_of] = recip[r_s]
        rr = rr.reshape(NCH, CH).T.astype(np.float32).copy()        # [128, NCH]
        rrec = rrec.reshape(NCH, CH).T.astype(np.float32).copy()
        # label -> node map (DUMMY = absent-on-core -> 0; DUMMY+1 = globally
        # isolated -> -b2/NCORES so the downstream +b2 cancels)
        labA = np.full(NP, DUMMY, np.int64)
        labA[:N][cnt[:N] == 0] = DUMMY + 1
        run_nodes = r_s[cc["starts"]]
        labA[run_nodes] = cc["lab_of_run"]
        labA_w = _wrap16(labA, 4, NP // 16)
        # node-product gather idx
        gidx = np.zeros((128, EPAD // 16), np.int16)
        gidx[0:64] = _wrap16(r_slot, 4, EPAD // 16)
        gidx[64:128] = _wrap16(c_slot, 4, EPAD // 16)
        # pair head: slot of local pair p and its partner
        pos = np.empty(EC, np.int64)
        pos[cc["order"]] = np.arange(EC)
        slot_pos = slot_of[pos]                  # original local edge -> slot
        ia = np.zeros(SP, np.int64)
        ib = np.zeros(SP, np.int64)
        ia[:S] = slot_pos[:S]
        ib[:S] = slot_pos[S:]
        qidxa = _wrap16(ia, 2, SP // 16)
        qidxb = _wrap16(ib, 2, SP // 16)
        # edge features (slot order, fp16, transposed)
        ea = np.zeros((EPAD, 644), np.float16)
        ea[slot_of, :640] = edge_attr[ids_s].astype(np.float16)
        ea[slot_of, 640:] = edge_rot[ids_s].astype(np.float16)
        eaT = np.ascontiguousarray(ea.T)
        per_core.append(dict(eaT=eaT, gidx=gidx, qidxa=qidxa, qidxb=qidxb,
                             rr=rr, rrec=rrec, labA=labA_w))

    # node features transposed fp16 [260, NP]
    nfT = np.zeros((260, NP), np.float16)
    nfT[:, :N] = np.concatenate([x_org, x_rot], axis=1).astype(np.float16).T

    w = weights
    f16 = lambda a: np.ascontiguousarray(a, dtype=np.float16)
    col2 = lambda a: np.ascontiguousarray(np.asarray(a, np.float32).reshape(-1, 1))

    def pack_chunks(wm):  # [K,64] -> [128, ceil(K/128)*64]
        K = wm.shape[0]
        nch = (K + 127) // 128
        out = np.zeros((128, nch * 64), np.float16)
        for j in range(nch):
            blk = wm[128 * j: min(128 * (j + 1), K)]
            out[: blk.shape[0], 64 * j: 64 * j + 64] = blk
        return out

    shared = dict(
        nfT=nfT, iota=iota,
        wf=f16(np.concatenate([w["c1_w1"][520:1160], w["c2_w1"][128:768]], axis=1)),
        wrot=f16(w["c1_w1"][1160:1164]),
        wa1=pack_chunks(f16(w["c1_w1"][0:260])),
        wb1=pack_chunks(f16(w["c1_w1"][260:520])),
        wa2=f16(w["c2_w1"][0:64]), wb2=f16(w["c2_w1"][64:128]),
        wa3=f16(np.concatenate([w["c3_w1"][64:128], w["c3_w1"][0:64]])),
        wb3=f16(np.concatenate([w["c3_w1"][192:256], w["c3_w1"][128:192]])),
        wa4=f16(w["c4_w1"][0:128]), wb4=f16(w["c4_w1"][128:256]),
        we2=f16(w["c2_w1"][768:832]),
        we3=f16(np.concatenate([w["c3_w1"][320:384], w["c3_w1"][256:320]])),
        we4=f16(w["c4_w1"][256:384]),
        w2_1=f16(w["c1_w2"]), w2_2=f16(w["c2_w2"]),
        w2_3=f16(w["c3_w2"]), w2_4=f16(w["c4_w2"]),
        b1_1=col2(w["c1_b1"]), b1_2=col2(w["c2_b1"]),
        b1_3=col2(w["c3_b1"]), b1_4=col2(w["c4_b1"]),
        b2_1=col2(w["c1_b2"]), b2_2=col2(w["c2_b2"]),
        b2_3=col2(w["c3_b2"]), b2_4=col2(w["c4_b2"]),

        lin1w=f16(w["lin1_w"]), lin1b=col2(w["lin1_b"]),
        efw1=f16(w["efc_w1"]), efb1=col2(w["efc_b1"]), efw2=f16(w["efc_w2"]),
        ones4=f16(np.ones((4, 4))),
    )
    meta = dict(N=N, E=E, S=S, EC=EC, EPAD=EPAD, NT=NT, NCH=NCH, NWIN=NWIN,
                NLAB=NLAB, NLABP=NLABP, NP=NP, NPCH=NPCH, SP=SP,
                efb2=float(np.asarray(w["efc_b2"]).reshape(-1)[0]))
    return shared, per_core, meta


def _build(meta):
    m = meta
    NP, EPAD, NT, NCH, NWIN, NLAB, NLABP, SP, NPCH = (
        m["NP"], m["EPAD"], m["NT"], m["NCH"], m["NWIN"], m["NLAB"],
        m["NLABP"], m["SP"], m["NPCH"])
    nc = bacc.Bacc(None, target_bir_lowering=False, debug=False)

    def di(n, s, d=F16):
        return nc.dram_tensor(n, s, d, kind="ExternalInput")

    eaT = di("eaT", [644, EPAD])
    nfT = di("nfT", [260, NP])
    iota_d = di("iota", [128, WN])
    gidx_d = di("gidx", [128, EPAD // 16], I16)
    qidxa_d = di("qidxa", [32, SP // 16], I16)
    qidxb_d = di("qidxb", [32, SP // 16], I16)
    rr_d = di("rr", [128, NCH], F32)
    rrec_d = di("rrec", [128, NCH], F32)
    labA_d = di("labA", [64, NP // 16], I16)
    wf_d = di("wf", [640, 128]); wrot_d = di("wrot", [4, 64])
    wa1_d = di("wa1", [128, 192]); wb1_d = di("wb1", [128, 192])
    wa2_d = di("wa2", [64, 64]); wb2_d = di("wb2", [64, 64])
    wa3_d = di("wa3", [128, 64]); wb3_d = di("wb3", [128, 64])
    wa4_d = di("wa4", [128, 64]); wb4_d = di("wb4", [128, 64])
    we2_d = di("we2", [64, 64]); we3_d = di("we3", [128, 64]); we4_d = di("we4", [128, 64])
    w2_d = [di(f"w2_{k}", [64, 64]) for k in (1, 2, 3, 4)]
    b1_d = [di(f"b1_{k}", [64, 1], F32) for k in (1, 2, 3, 4)]
    b2_d = [di(f"b2_{k}", [64, 1], F32) for k in (1, 2, 3, 4)]
    lin1w_d = di("lin1w", [64, 4]); lin1b_d = di("lin1b", [4, 1], F32)
    efw1_d = di("efw1", [64, 32]); efb1_d = di("efb1", [32, 1], F32)
    efw2_d = di("efw2", [32, 1]); ones4_d = di("ones4", [4, 4])

    out_x = nc.dram_tensor("out_x", [4, NP], F32, kind="ExternalOutput")
    out_p = nc.dram_tensor("out_p", [1, SP], F32, kind="ExternalOutput")
    t2d = nc.dram_tensor("t2d", [64, EPAD], F16)

    with tile.TileContext(nc) as tc:
        nc.gpsimd.load_library(library_config.ap_gather)
        import contextlib
        stack = contextlib.ExitStack()
        sb = stack.enter_context(tc.tile_pool(name="sb", bufs=1))
        wk = stack.enter_context(tc.tile_pool(name="wk", bufs=2))
        pm = stack.enter_context(tc.tile_pool(name="pm", bufs=2, space="PSUM"))
        pa = stack.enter_context(tc.tile_pool(name="pa", bufs=2, space="PSUM"))
        pb = stack.enter_context(tc.tile_pool(name="pb", bufs=2, space="PSUM"))
        pw = stack.enter_context(tc.tile_pool(name="pw", bufs=2, space="PSUM"))
        dr = stack.enter_context(tc.tile_pool(name="dr", bufs=1, space="DRAM"))

        # ---- persistent SBUF ----
        P_ab = sb.tile([128, NP], F32)
        NX = sb.tile([128, NP], F16)
        EX = sb.tile([128, EPAD], F16)
        XD = sb.tile([64, NLABP], F32)   # dense label sums
        t_iota = sb.tile([128, WN], F16)
        t_gidx = sb.tile([128, EPAD // 16], I16)
        t_qidxa = sb.tile([32, SP // 16], I16)
        t_qidxb = sb.tile([32, SP // 16], I16)
        t_rr = sb.tile([128, NCH], F32)
        t_rrec = sb.tile([128, NCH], F32)
        t_labA = sb.tile([64, NP // 16], I16)
        t_wf = sb.tile([128, 5 * 128], F16)
        t_wrot = sb.tile([4, 64], F16)
        t_wa1 = sb.tile([128, 192], F16); t_wb1 = sb.tile([128, 192], F16)
        t_wa2 = sb.tile([64, 64], F16); t_wb2 = sb.tile([64, 64], F16)
        t_wa3 = sb.tile([128, 64], F16); t_wb3 = sb.tile([128, 64], F16)
        t_wa4 = sb.tile([128, 64], F16); t_wb4 = sb.tile([128, 64], F16)
        t_we2 = sb.tile([64, 64], F16); t_we3 = sb.tile([128, 64], F16)
        t_we4 = sb.tile([128, 64], F16)
        t_w2 = [sb.tile([64, 64], F16, name=f"t_w2_{k}") for k in range(4)]
        t_b1 = [sb.tile([64, 1], F32, name=f"t_b1_{k}") for k in range(4)]
        t_b2 = [sb.tile([64, 1], F32, name=f"t_b2_{k}") for k in range(4)]
        t_lin1w = sb.tile([64, 4], F16); t_lin1b = sb.tile([4, 1], F32)
        t_efw1 = sb.tile([64, 32], F16); t_efb1 = sb.tile([32, 1], F32)
        t_efw2 = sb.tile([32, 1], F16); t_ones4 = sb.tile([4, 4], F16)

        loads = [(t_iota, iota_d), (t_gidx, gidx_d), (t_qidxa, qidxa_d),
                 (t_qidxb, qidxb_d),
                 (t_rr, rr_d), (t_rrec, rrec_d), (t_labA, labA_d),
                 (t_wrot, wrot_d), (t_wa1, wa1_d), (t_wb1, wb1_d),
                 (t_wa2, wa2_d), (t_wb2, wb2_d), (t_wa3, wa3_d), (t_wb3, wb3_d),
                 (t_wa4, wa4_d), (t_wb4, wb4_d), (t_we2, we2_d), (t_we3, we3_d),
                 (t_we4, we4_d), (t_lin1w, lin1w_d), (t_lin1b, lin1b_d),
                 (t_efw1, efw1_d), (t_efb1, efb1_d), (t_efw2, efw2_d),
                 (t_ones4, ones4_d)]
        for k in range(4):
            loads += [(t_w2[k], w2_d[k]), (t_b1[k], b1_d[k]), (t_b2[k], b2_d[k])]
        for dst, src in loads:
            nc.sync.dma_start(out=dst[:], in_=src[:])
        for j in range(5):
            nc.sync.dma_start(out=t_wf[:, 128 * j:128 * (j + 1)],
                              in_=wf_d[128 * j:128 * (j + 1), :])
        nc.vector.memset(XD[:, NLAB:], 0.0)   # dummy zero region

        # ================= conv k =================
        def conv(k):
            # node products P_a | P_b
            for j in range(NPCH):
                cs = slice(T * j, T * (j + 1))
                pp = pa.tile([64, T], F32, name="pp", tag="pa")
                pq = pb.tile([64, T], F32, name="pq", tag="pb")
                if k == 0:
                    nfa = wk.tile([128, T], F16, name="nfa", tag="ea0")
                    nfb = wk.tile([128, T], F16, name="nfb", tag="ea1")
                    nfc = wk.tile([4, T], F16, name="nfc", tag="rot")
                    nc.sync.dma_start(out=nfa[:], in_=nfT[0:128, cs])
                    nc.sync.dma_start(out=nfb[:], in_=nfT[128:256, cs])
                    nc.sync.dma_start(out=nfc[:], in_=nfT[256:260, cs])
                    for wt, pr in ((t_wa1, pp), (t_wb1, pq)):
                        nc.tensor.matmul(pr[:], wt[:, 0:64], nfa[:], start=True, stop=False)
                        nc.tensor.matmul(pr[:], wt[:, 64:128], nfb[:], start=False, stop=False)
                        nc.tensor.matmul(pr[:], wt[0:4, 128:192], nfc[:], start=False, stop=True)
                elif k == 1:
                    nc.tensor.matmul(pp[:], t_wa2[:], NX[0:64, cs], start=True, stop=True)
                    nc.tensor.matmul(pq[:], t_wb2[:], NX[0:64, cs], start=True, stop=True)
                elif k == 2:
                    nc.tensor.matmul(pp[:], t_wa3[:], NX[:, cs], start=True, stop=True)
                    nc.tensor.matmul(pq[:], t_wb3[:], NX[:, cs], start=True, stop=True)
                else:
                    nc.tensor.matmul(pp[:], t_wa4[:], NX[:, cs], start=True, stop=True)
                    nc.tensor.matmul(pq[:], t_wb4[:], NX[:, cs], start=True, stop=True)
                nc.scalar.copy(P_ab[0:64, cs], pp[:])
                nc.scalar.copy(P_ab[64:128, cs], pq[:])

            # edge loop
            for t in range(NT):
                es = slice(T * t, T * (t + 1))
                if k == 0:
                    ps_m = pm.tile([128, T], F32, name="ps_m", tag="pm")
                    ea = [wk.tile([128, T], F16, name=f"ea{j}", tag=f"ea{j}")
                          for j in range(5)]
                    rot = wk.tile([4, T], F16, name="rot", tag="rot")
                    for j in range(5):
                        nc.sync.dma_start(out=ea[j][:], in_=eaT[128 * j:128 * (j + 1), es])
                    nc.sync.dma_start(out=rot[:], in_=eaT[640:644, es])
                    for j in range(5):
                        nc.tensor.matmul(ps_m[:], t_wf[:, 128 * j:128 * (j + 1)], ea[j][:],
                                         start=(j == 0), stop=(j == 4))
                    nc.tensor.matmul(ps_m[0:64, :], t_wrot[:], rot[:],
                                     start=False, stop=True, skip_group_check=True)
                    t2c = wk.tile([64, T], F16, name="t2c", tag="t2c")
                    nc.scalar.copy(t2c[:], ps_m[64:128, :])
                    nc.sync.dma_start(out=t2d[:, es], in_=t2c[:])
                    mrows = ps_m[0:64, :]
                else:
                    ps_m = pm.tile([64, T], F32, name="ps_m", tag="pm")
                    if k == 1:
                        nc.tensor.matmul(ps_m[:], t_we2[:], EX[0:64, es], start=True, stop=True)
                    elif k == 2:
                        nc.tensor.matmul(ps_m[:], t_we3[:], EX[:, es], start=True, stop=True)
                    else:
                        nc.tensor.matmul(ps_m[:], t_we4[:], EX[:, es], start=True, stop=True)
                    mrows = ps_m[:]

                g = wk.tile([128, T], F32, name="g", tag="g")
                nc.gpsimd.ap_gather(g[:], P_ab[:], t_gidx[:, 32 * t:32 * (t + 1)],
                                    channels=128, num_elems=NP, d=1, num_idxs=T)
                # mixed-space adds (PSUM + SB) dodge the same-base-partition rule
                gtmp = wk.tile([64, T], F16, name="gtmp", tag="gs")
                nc.vector.tensor_add(out=gtmp[:], in0=mrows, in1=g[64:128, :])
                hpre = wk.tile([64, T], F16, name="hpre", tag="hpre")
                if k == 1:
                    t2r = wk.tile([64, T], F16, name="t2r", tag="t2c")
                    nc.sync.dma_start(out=t2r[:], in_=t2d[:, es])
                    gt2 = wk.tile([64, T], F16, name="gt2", tag="gt2")
                    nc.vector.scalar_tensor_tensor(out=gt2[:], in0=gtmp[:],
                                                   scalar=t_b1[k][:], in1=g[0:64, :],
                                                   op0=ALU.add, op1=ALU.add)
                    nc.vector.tensor_add(out=hpre[:], in0=gt2[:], in1=t2r[:])
                else:
                    nc.vector.scalar_tensor_tensor(out=hpre[:], in0=gtmp[:],
                                                   scalar=t_b1[k][:], in1=g[0:64, :],
                                                   op0=ALU.add, op1=ALU.add)
                h = wk.tile([64, T], F16, name="h", tag="h")
                nc.scalar.activation(h[:], hpre[:], AF.Relu)

                # (A) feature-major conv output -> EX
                ps_a = pa.tile([64, T], F32, name="ps_a", tag="pa")
                nc.tensor.matmul(ps_a[:], t_w2[k][:], h[:], start=True, stop=True)
                exr = slice(0, 64) if k in (0, 2) else slice(64, 128)
                if k < 3:
                    nc.scalar.activation(EX[exr, es], ps_a[:], AF.Relu, bias=t_b2[k][:])
                else:
                    nc.vector.tensor_scalar(out=EX[exr, es], in0=ps_a[:],
                                            scalar1=t_b2[k][:], scalar2=None, op0=ALU.add)

                # (B) edge-major m (b2 folded past the mean)
                ps_b = pb.tile([128, 256], F32, name="ps_b", tag="pb")
                for ci in range(4):
                    nc.tensor.matmul(ps_b[:, 64 * ci:64 * (ci + 1)],
                                     h[:, 128 * ci:128 * (ci + 1)], t_w2[k][:],
                                     start=True, stop=True)
                mem = wk.tile([128, 256], F16, name="mem", tag="mem")
                nc.scalar.copy(mem[:], ps_b[:])

                # scatter into label space (windows of 2 chunks)
                for ci in range(4):
                    ch = 4 * t + ci
                    wnd = ch // 2
                    if ch % 2 == 0:
                        conv.ps_w = pw.tile([64, WN], F32, name="ps_w", tag="pw")
                    ps_w = conv.ps_w
                    s_t = wk.tile([128, WN], F16, name="s_t", tag="s_t")
                    nc.vector.tensor_scalar(out=s_t[:], in0=t_iota[:],
                                            scalar1=t_rr[:, ch:ch + 1],
                                            scalar2=t_rrec[:, ch:ch + 1],
                                            op0=ALU.is_equal, op1=ALU.mult)
                    nc.tensor.matmul(ps_w[:], mem[:, 64 * ci:64 * (ci + 1)], s_t[:],
                                     start=(ch % 2 == 0), stop=(ch % 2 == 1))
                    if ch % 2 == 1:
                        nc.scalar.copy(XD[:, WN * wnd:WN * (wnd + 1)], ps_w[:])

            # un-permute to global node order, stage for AllReduce
            nc.vector.tensor_scalar(out=XD[:, NLAB + 1:NLAB + 2], in0=t_b2[k][:],
                                    scalar1=-1.0 / NCORES, scalar2=None, op0=ALU.mult)
            ar_in = dr.tile([64, NP], F16, name=f"ar_in{k}")
            ar_out = dr.tile([64, NP], F16, name=f"ar_out{k}", addr_space="Shared")
            for j in range(NPCH):
                cs = slice(T * j, T * (j + 1))
                ga = wk.tile([64, T], F32, name="ga", tag="ga")
                nc.gpsimd.ap_gather(ga[:], XD[:, :NLABP], t_labA[:, 32 * j:32 * (j + 1)],
                                    channels=64, num_elems=NLABP, d=1, num_idxs=T)
                gc = wk.tile([64, T], F16, name="gc", tag="gs")
                nc.vector.tensor_copy(out=gc[:], in_=ga[:])
                nc.sync.dma_start(out=ar_in[:, cs], in_=gc[:])
            nc.gpsimd.collective_compute(
                "AllReduce", ALU.add,
                replica_groups=[list(range(NCORES))],
                ins=[ar_in[:]], outs=[ar_out[:]],
            )
            # mean(+b2)(+relu) -> NX
            nxr = slice(0, 64) if k in (0, 2, 3) else slice(64, 128)
            for j in range(NPCH):
                cs = slice(T * j, T * (j + 1))
                xr = wk.tile([64, T], F16, name="xr", tag="ga")
                nc.sync.dma_start(out=xr[:], in_=ar_out[:, cs])
                if k == 0:
                    nc.vector.tensor_scalar(out=NX[nxr, cs], in0=xr[:],
                                            scalar1=t_b2[k][:], scalar2=None, op0=ALU.add)
                else:
                    nc.scalar.activation(NX[nxr, cs], xr[:], AF.Relu, bias=t_b2[k][:])

        conv.ps_w = None
        for k in range(4):
            conv(k)

        # ================= head: x =================
        for j in range(NPCH):
            cs = slice(T * j, T * (j + 1))
            ps_y = pa.tile([4, T], F32, name="ps_y", tag="pa")
            nc.tensor.matmul(ps_y[:], t_lin1w[:], NX[0:64, cs], start=True, stop=True)
            ysb = wk.tile([4, T], F32, name="ysb", tag="g")
            nc.vector.tensor_scalar(out=ysb[:], in0=ps_y[:], scalar1=t_lin1b[:],
                                    scalar2=None, op0=ALU.add)
            sq = wk.tile([4, T], F16, name="sq", tag="gs")
            nc.vector.tensor_tensor(out=sq[:], in0=ysb[:], in1=ysb[:], op=ALU.mult)
            ps_s = pb.tile([4, T], F32, name="ps_s", tag="pb")
            nc.tensor.matmul(ps_s[:], t_ones4[:], sq[:], start=True, stop=True)
            rt = wk.tile([4, T], F32, name="rt", tag="ga")
            nc.scalar.activation(rt[:], ps_s[:], AF.Sqrt)
            nc.vector.tensor_scalar_max(out=rt[:], in0=rt[:], scalar1=1e-12)
            nc.vector.reciprocal(out=rt[:], in_=rt[:])
            yo = wk.tile([4, T], F16, name="yo", tag="hpre")
            nc.vector.tensor_tensor(out=yo[:], in0=ysb[:], in1=rt[:], op=ALU.mult)
            nc.gpsimd.dma_start(out=out_x[:, cs], in_=yo[:])

        # ================= head: edge pairs =================
        qst = sb.tile([32, EPAD], F32, name="qst", tag="XD")
        for t in range(NT):
            es = slice(T * t, T * (t + 1))
            ps_q = pa.tile([32, T], F32, name="ps_q", tag="pa")
            ex4c = wk.tile([64, T], F16, name="ex4c", tag="gs")
            nc.scalar.copy(ex4c[:], EX[64:128, es])
            nc.tensor.matmul(ps_q[:], t_efw1[:], ex4c[:], start=True, stop=True)
            nc.scalar.copy(qst[:, es], ps_q[:])
        for t in range(SP // T):
            es = slice(T * t, T * (t + 1))
            ha = wk.tile([32, T], F32, name="ha", tag="g")
            hb = wk.tile([32, T], F32, name="hb", tag="ga")
            nc.gpsimd.ap_gather(ha[:], qst[:], t_qidxa[:, 32 * t:32 * (t + 1)],
                                channels=32, num_elems=EPAD, d=1, num_idxs=T)
            nc.gpsimd.ap_gather(hb[:], qst[:], t_qidxb[:, 32 * t:32 * (t + 1)],
                                channels=32, num_elems=EPAD, d=1, num_idxs=T)
            hs = wk.tile([32, T], F16, name="hs", tag="t2c")
            nc.vector.tensor_add(out=hs[:], in0=ha[:], in1=hb[:])
            hq = wk.tile([32, T], F16, name="hq", tag="mem")
            nc.scalar.activation(hq[:], hs[:], AF.Relu, bias=t_efb1[:])
            ps_p = pb.tile([1, T], F32, name="ps_p", tag="pb")
            nc.tensor.matmul(ps_p[:], t_efw2[:], hq[:], start=True, stop=True)
            po = wk.tile([1, T], F32, name="po", tag="rot")
            nc.vector.tensor_scalar(out=po[:], in0=ps_p[:], scalar1=m["efb2"],
                                    scalar2=None, op0=ALU.add)
            nc.sync.dma_start(out=out_p[:, es], in_=po[:])

        stack.close()
    return nc


def kernel(**inputs):
    x_org = np.asarray(inputs["x_org"], np.float32)
    x_rot = np.asarray(inputs["x_rot"], np.float32)
    edge_index = np.asarray(inputs["edge_index"])
    edge_attr = np.asarray(inputs["edge_attr"], np.float32)
    edge_rot = np.asarray(inputs["edge_rot"], np.float32)
    weights = {k: np.asarray(v, np.float32) for k, v in inputs.items()
               if k not in ("x_org", "x_rot", "edge_index", "edge_attr", "edge_rot")}

    shared, per_core, meta = _host_prep(x_org, x_rot, edge_index, edge_attr,
                                        edge_rot, weights)
    nc = _build(meta)
    nc.finalize()
    in_maps = [{**shared, **pc} for pc in per_core]
    res = run_bass_kernel_spmd(nc, in_maps, core_ids=list(range(NCORES)))
    global LAST_RESULT
    LAST_RESULT = res
    r0 = res.results[0]
    N, S = meta["N"], meta["S"]
    x = np.ascontiguousarray(r0["out_x"][:, :N].T).astype(np.float32)
    pred = np.concatenate([res.results[c]["out_p"][0, :S] for c in range(NCORES)])
    return x, pred.reshape(-1, 1).astype(np.float32)
